# revision 7
# baseline (speedup 1.0000x reference)
import numpy as np

import concourse.bass as bass
import concourse.mybir as mybir
import concourse.tile as tile
from concourse import bacc
from concourse.bass_utils import run_bass_kernel_spmd

B, T, D, H = 2, 512, 512, 8
HD = D // H
FF = 4 * D
MOM_BETA, NOV_BETA, NOV_GAIN = 0.9, 0.95, 3.0
LR_MIN, LR_MAX = 0.5, 3.0
P = 128
NT = T // P
ND = D // P
NF = FF // P
N_CORES = 8
GROUPS = [[0, 1, 2, 3], [4, 5, 6, 7]]
EPS = 1e-5
SCALE = HD ** -0.5
NEG = -1e30

F32 = mybir.dt.float32
F32R = mybir.dt.float32r
AF = mybir.ActivationFunctionType
ALU = mybir.AluOpType
AX = mybir.AxisListType

MM_MODE = "f32"

IN_SPECS = [
    ("x", [T, D]), ("wqT", [D, P]), ("wk1T", [D, P]), ("wk2T", [D, P]),
    ("wvT", [D, P]), ("woT", [P, D]), ("lam_l", [1, 2]), ("lngb", [6, D]),
    ("baseT", [D, P]), ("bias_sl", [1, P]), ("loglr", [1, 1]),
    ("aemaT", [T, T]), ("alrmT", [T, T]), ("c0", [T, 1]), ("bmat", [T, T]),
    ("lstrictT", [T, T]), ("triu01", [P, P]), ("idmat", [P, P]),
    ("maskdiag", [P, P]), ("hx_sel", [D, P]), ("tsel", [T, P]),
    ("w1T", [D, FF]), ("b1c", [P, NF]), ("w2T", [FF, D]), ("b2", [1, D]),
]


def _mm(ap):
    return ap.bitcast(F32R) if MM_MODE == "f32r" else ap


def build_nc():
    nc = bacc.Bacc("TRN2", target_bir_lowering=False, debug=False,
                   num_devices=N_CORES)
    io = {name: nc.dram_tensor(name, shape, F32, kind="ExternalInput").ap()
          for name, shape in IN_SPECS}
    io["out"] = nc.dram_tensor("out", [P, D], F32, kind="ExternalOutput").ap()
    with tile.TileContext(nc) as tc:
        _body(nc, tc, io)
    nc.compile()
    return nc



class _Pool:

    def __init__(self, tc, **kw):
        self._cm = tc.tile_pool(**kw)
        self.pool = self._cm.__enter__()

    def tile(self, *a, **kw):
        kw.setdefault("name", kw.get("tag") or "t")
        return self.pool.tile(*a, **kw)

    def close(self):
        self._cm.__exit__(None, None, None)


def _body(nc, tc, io):
    persist = _Pool(tc, name="persist", bufs=1)
    psA = _Pool(tc, name="psA", bufs=3, space="PSUM")
    psB = _Pool(tc, name="psB", bufs=4, space="PSUM")
    dram = _Pool(tc, name="dram", bufs=1, space="DRAM")

    def pbig():
        return psA.tile([P, T], F32, tag="pb")

    def psmall():
        return psB.tile([P, P], F32, tag="ps")

    id_t = persist.tile([P, P], F32, tag="id_t")
    nc.sync.dma_start(id_t[:], io["idmat"][:])
    eps_t = persist.tile([P, 1], F32, tag="eps_t")
    nc.vector.memset(eps_t[:], EPS)
    gb = []
    for i in range(6):
        t = persist.tile([P, D], F32, tag=f"lngb{i}")
        nc.sync.dma_start(t[:], io["lngb"][i:i + 1, :].to_broadcast((P, D)))
        gb.append(t)

    def layernorm_tile(dst, src, g_t, b_t, pool):
        stats = pool.tile([P, 6], F32, tag="ln_stats")
        mv = pool.tile([P, 2], F32, tag="ln_mv")
        nc.vector.bn_stats(out=stats[:], in_=src[:])
        nc.vector.bn_aggr(out=mv[:], in_=stats[:])
        rstd = pool.tile([P, 1], F32, tag="ln_rstd")
        nc.scalar.activation(out=rstd[:], in_=mv[:, 1:2], func=AF.Sqrt,
                             bias=eps_t[:], scale=1.0)
        nc.vector.reciprocal(out=rstd[:], in_=rstd[:])
        nc.vector.tensor_scalar(out=dst[:], in0=src[:],
                                scalar1=mv[:, 0:1], scalar2=rstd[:],
                                op0=ALU.subtract, op1=ALU.mult)
        nc.vector.tensor_mul(dst[:], dst[:], g_t[:])
        nc.vector.tensor_add(dst[:], dst[:], b_t[:])

    def transpose_to(dst_tiles, src_tiles):
        for dt_ in range(ND):
            pt = pbig()
            for tt in range(NT):
                nc.tensor.transpose(pt[:, tt * P:(tt + 1) * P],
                                    src_tiles[tt][:, dt_ * P:(dt_ + 1) * P],
                                    id_t[:])
            nc.vector.tensor_copy(dst_tiles[dt_][:], pt[:])

    attn = _Pool(tc, name="attn", bufs=1)
    atmp = _Pool(tc, name="atmp", bufs=2)

    x_t = [persist.tile([P, D], F32, tag=f"x{i}") for i in range(NT)]
    for i in range(NT):
        nc.sync.dma_start(x_t[i][:], io["x"][i * P:(i + 1) * P, :])

    xln = [attn.tile([P, D], F32, tag=f"xln{i}") for i in range(NT)]
    for i in range(NT):
        layernorm_tile(xln[i], x_t[i], gb[0], gb[1], atmp)

    xlnT = [attn.tile([P, T], F32, tag=f"xlnT{i}") for i in range(ND)]
    transpose_to(xlnT, xln)

    lam_t = attn.tile([P, 2], F32, tag="lam_t")
    nc.sync.dma_start(lam_t[:], io["lam_l"][0:1, :].to_broadcast((P, 2)))
    nc.scalar.activation(out=lam_t[:], in_=lam_t[:], func=AF.Sigmoid)

    wq_t = attn.tile([P, ND, P], F32, tag="wq_t")
    wk1_t = attn.tile([P, ND, P], F32, tag="wk1_t")
    wk2_t = attn.tile([P, ND, P], F32, tag="wk2_t")
    wv_t = attn.tile([P, ND, P], F32, tag="wv_t")
    for w_sb, name in ((wq_t, "wqT"), (wk1_t, "wk1T"), (wk2_t, "wk2T"),
                       (wv_t, "wvT")):
        nc.sync.dma_start(w_sb[:], io[name].rearrange("(a b) c -> b a c", b=P))

    qT = attn.tile([P, T], F32, tag="qT")
    k1T = attn.tile([P, T], F32, tag="k1T")
    k2T = attn.tile([P, T], F32, tag="k2T")
    for dst, w_sb in ((qT, wq_t), (k1T, wk1_t), (k2T, wk2_t)):
        pt = pbig()
        for dt_ in range(ND):
            nc.tensor.matmul(pt[:], _mm(w_sb[:, dt_]), _mm(xlnT[dt_][:]),
                             start=(dt_ == 0), stop=(dt_ == ND - 1))
        nc.vector.tensor_copy(dst[:], pt[:])

    v_t = [attn.tile([P, P], F32, tag=f"v{i}") for i in range(NT)]
    for tt in range(NT):
        pt = psmall()
        for dt_ in range(ND):
            nc.tensor.matmul(pt[:], _mm(xlnT[dt_][:, tt * P:(tt + 1) * P]),
                             _mm(wv_t[:, dt_]),
                             start=(dt_ == 0), stop=(dt_ == ND - 1))
        nc.vector.tensor_copy(v_t[tt][:], pt[:])

    mskd = attn.tile([P, P], F32, tag="mskd")
    nc.sync.dma_start(mskd[:], io["maskdiag"][:])

    o2T = attn.tile([P, T], F32, tag="o2T")
    for h in range(2):
        ho = h * HD
        pt_sb = {}
        for qi in range(NT):
            nk = (qi + 1) * P
            s1 = pbig()
            s2 = pbig()
            nc.tensor.matmul(s1[:, :nk],
                             _mm(qT[ho:ho + HD, qi * P:(qi + 1) * P]),
                             _mm(k1T[ho:ho + HD, :nk]), start=True, stop=True)
            nc.tensor.matmul(s2[:, :nk],
                             _mm(qT[ho:ho + HD, qi * P:(qi + 1) * P]),
                             _mm(k2T[ho:ho + HD, :nk]), start=True, stop=True)
            nc.vector.tensor_add(s1[:, qi * P:nk], s1[:, qi * P:nk], mskd[:])
            nc.vector.tensor_add(s2[:, qi * P:nk], s2[:, qi * P:nk], mskd[:])
            e1 = atmp.tile([P, T], F32, tag="e1")
            e2 = atmp.tile([P, T], F32, tag="e2")
            sum1 = atmp.tile([P, 1], F32, tag="sum1")
            sum2 = atmp.tile([P, 1], F32, tag="sum2")
            nc.scalar.activation(out=e1[:, :nk], in_=s1[:, :nk], func=AF.Exp,
                                 scale=SCALE, accum_out=sum1[:])
            nc.scalar.activation(out=e2[:, :nk], in_=s2[:, :nk], func=AF.Exp,
                                 scale=SCALE, accum_out=sum2[:])
            rn1 = atmp.tile([P, 1], F32, tag="rn1")
            rn2 = atmp.tile([P, 1], F32, tag="rn2")
            nc.vector.reciprocal(rn1[:], sum1[:])
            nc.vector.reciprocal(rn2[:], sum2[:])
            nc.vector.tensor_mul(rn2[:], rn2[:], lam_t[:, h:h + 1])
            pmat = atmp.tile([P, T], F32, tag="pmat")
            nc.vector.tensor_scalar_mul(pmat[:, :nk], e1[:, :nk], rn1[:])
            nc.vector.tensor_scalar_mul(e2[:, :nk], e2[:, :nk], rn2[:])
            nc.vector.tensor_sub(pmat[:, :nk], pmat[:, :nk], e2[:, :nk])
            rabs = atmp.tile([P, 1], F32, tag="rabs")
            nc.vector.tensor_reduce(out=rabs[:], in_=pmat[:, :nk], axis=AX.X,
                                    op=ALU.add, apply_absolute_value=True)
            nc.vector.tensor_scalar_max(rabs[:], rabs[:], 1e-6)
            nc.vector.reciprocal(rabs[:], rabs[:])
            nc.vector.tensor_scalar_mul(pmat[:, :nk], pmat[:, :nk], rabs[:])
            ptp = pbig()
            for kt in range(qi + 1):
                nc.tensor.transpose(ptp[:, kt * P:(kt + 1) * P],
                                    pmat[:, kt * P:(kt + 1) * P], id_t[:])
            for kt in range(qi + 1):
                sb = attn.tile([P, P], F32, tag=f"ptT{h}_{kt}_{qi}")
                pt_sb[(kt, qi)] = sb
                nc.vector.tensor_copy(sb[:], ptp[:, kt * P:(kt + 1) * P])
        for qi in range(NT):
            pav = psmall()
            for kt in range(qi + 1):
                nc.tensor.matmul(pav[:HD, :], _mm(v_t[kt][:, ho:ho + HD]),
                                 _mm(pt_sb[(kt, qi)][:]),
                                 start=(kt == 0), stop=(kt == qi))
            nc.vector.tensor_copy(o2T[ho:ho + HD, qi * P:(qi + 1) * P],
                                  pav[:HD, :])

    wo_t = attn.tile([P, D], F32, tag="wo_t")
    nc.sync.dma_start(wo_t[:], io["woT"][:])
    cc_in = dram.tile([T, D], F32, tag="cc_in")
    for tt in range(NT):
        pt = pbig()
        nc.tensor.matmul(pt[:], _mm(o2T[:, tt * P:(tt + 1) * P]), _mm(wo_t[:]),
                         start=True, stop=True)
        ap_sb = atmp.tile([P, D], F32, tag="attnp")
        nc.vector.tensor_copy(ap_sb[:], pt[:])
        nc.sync.dma_start(cc_in[tt * P:(tt + 1) * P, :], ap_sb[:])

    cc_out = dram.tile([T, D], F32, tag="cc_out")
    nc.gpsimd.collective_compute(
        "AllReduce", ALU.add, replica_groups=GROUPS,
        ins=[cc_in.opt()], outs=[cc_out.opt()])

    h1 = [persist.tile([P, D], F32, tag=f"h1_{i}") for i in range(NT)]
    for i in range(NT):
        asum = atmp.tile([P, D], F32, tag="asum")
        nc.sync.dma_start(asum[:], cc_out[i * P:(i + 1) * P, :])
        nc.vector.tensor_add(h1[i][:], x_t[i][:], asum[:])

    atmp.close()
    attn.close()

    ttt = _Pool(tc, name="ttt", bufs=1)
    ttmp = _Pool(tc, name="ttmp", bufs=2)
    ttc = _Pool(tc, name="ttc", bufs=1)

    x2 = [ttt.tile([P, D], F32, tag=f"x2_{i}") for i in range(NT)]
    for i in range(NT):
        layernorm_tile(x2[i], h1[i], gb[2], gb[3], ttmp)
    x2T = [ttt.tile([P, T], F32, tag=f"x2T{i}") for i in range(ND)]
    transpose_to(x2T, x2)

    kg = [ttc.tile([P, T], F32, tag=f"kg{i}") for i in range(NT)]
    for ti in range(NT):
        pt = pbig()
        for dt_ in range(ND):
            nc.tensor.matmul(pt[:], _mm(x2T[dt_][:, ti * P:(ti + 1) * P]),
                             _mm(x2T[dt_][:]),
                             start=(dt_ == 0), stop=(dt_ == ND - 1))
        nc.vector.tensor_copy(kg[ti][:], pt[:])

    aema_t = ttc.tile([P, NT, T], F32, tag="aema_t")
    nc.sync.dma_start(aema_t[:], io["aemaT"].rearrange("(a b) c -> b a c", b=P))
    lsc = [ttmp.tile([P, 1], F32, tag=f"lsc{i}") for i in range(NT)]
    for ti in range(NT):
        pt = pbig()
        for st in range(NT):
            nc.tensor.matmul(pt[:], _mm(aema_t[:, st, ti * P:(ti + 1) * P]),
                             _mm(x2[st][:]),
                             start=(st == 0), stop=(st == NT - 1))
        diff = ttmp.tile([P, D], F32, tag="ediff")
        nc.vector.tensor_sub(diff[:], x2[ti][:], pt[:])
        nov = ttmp.tile([P, 1], F32, tag="nov")
        nc.vector.tensor_reduce(out=nov[:], in_=diff[:], axis=AX.X,
                                op=ALU.add, apply_absolute_value=True)
        nc.vector.tensor_scalar(out=lsc[ti][:], in0=nov[:],
                                scalar1=NOV_GAIN / D, scalar2=1.0,
                                op0=ALU.mult, op1=ALU.add)
        nc.vector.tensor_scalar_max(lsc[ti][:], lsc[ti][:], LR_MIN)
        nc.vector.tensor_scalar_min(lsc[ti][:], lsc[ti][:], LR_MAX)

    lr_t = ttmp.tile([P, 1], F32, tag="lr_t")
    nc.sync.dma_start(lr_t[:], io["loglr"][0:1, :].to_broadcast((P, 1)))
    nc.scalar.activation(out=lr_t[:], in_=lr_t[:], func=AF.Exp)
    nc.vector.tensor_scalar_max(lr_t[:], lr_t[:], 1e-5)
    nc.vector.tensor_scalar_min(lr_t[:], lr_t[:], 1.0)
    alrm_t = ttc.tile([P, NT, T], F32, tag="alrm_t")
    nc.sync.dma_start(alrm_t[:], io["alrmT"].rearrange("(a b) c -> b a c", b=P))
    s_t = [ttmp.tile([P, 1], F32, tag=f"s{i}") for i in range(NT)]
    for ti in range(NT):
        pt = psmall()
        for st in range(NT):
            nc.tensor.matmul(pt[:, 0:1],
                             _mm(alrm_t[:, st, ti * P:(ti + 1) * P]),
                             _mm(lsc[st][:]),
                             start=(st == 0), stop=(st == NT - 1))
        c0s = ttmp.tile([P, 1], F32, tag="c0s")
        nc.sync.dma_start(c0s[:], io["c0"][ti * P:(ti + 1) * P, :])
        nc.vector.tensor_add(c0s[:], c0s[:], pt[:, 0:1])
        nc.vector.tensor_mul(s_t[ti][:], c0s[:], lr_t[:])

    bm_t = ttc.tile([P, NT, T], F32, tag="bm_t")
    nc.sync.dma_start(bm_t[:], io["bmat"].rearrange("(a b) c -> b a c", b=P))
    z_t = [bm_t[:, i] for i in range(NT)]
    for tt_ in range(NT):
        nc.vector.tensor_scalar_mul(z_t[tt_][:], bm_t[:, tt_], s_t[tt_][:])
    lst_t = ttc.tile([P, NT, T], F32, tag="lst_t")
    nc.sync.dma_start(lst_t[:],
                      io["lstrictT"].rearrange("(a b) c -> b a c", b=P))
    wT = [ttc.tile([P, T], F32, tag=f"wT{i}") for i in range(NT)]
    for st in range(NT):
        pt = pbig()
        for tt_ in range(NT):
            nc.tensor.matmul(pt[:], _mm(z_t[tt_][:, st * P:(st + 1) * P]),
                             _mm(lst_t[:, tt_]),
                             start=(tt_ == 0), stop=(tt_ == NT - 1))
        nc.vector.tensor_copy(wT[st][:], pt[:])

    tri_t = ttt.tile([P, P], F32, tag="tri_t")
    nc.sync.dma_start(tri_t[:], io["triu01"][:])
    mt = {}
    for jt in range(NT):
        for it in range(jt, NT):
            m = ttt.tile([P, P], F32, tag=f"mt{jt}_{it}")
            nc.vector.tensor_mul(m[:], wT[jt][:, it * P:(it + 1) * P],
                                 kg[jt][:, it * P:(it + 1) * P])
            if jt == it:
                nc.vector.tensor_mul(m[:], m[:], tri_t[:])
            mt[(jt, it)] = m

    base_t = ttt.tile([P, ND, P], F32, tag="base_t")
    nc.sync.dma_start(base_t[:], io["baseT"].rearrange("(a b) c -> b a c", b=P))
    biasr = ttt.tile([P, P], F32, tag="biasr")
    nc.sync.dma_start(biasr[:], io["bias_sl"][0:1, :].to_broadcast((P, P)))
    e0 = []
    for ti in range(NT):
        pt = psmall()
        for dt_ in range(ND):
            nc.tensor.matmul(pt[:], _mm(x2T[dt_][:, ti * P:(ti + 1) * P]),
                             _mm(base_t[:, dt_]),
                             start=(dt_ == 0), stop=(dt_ == ND - 1))
        e = ttt.tile([P, P], F32, tag=f"e0_{ti}")
        nc.vector.tensor_add(e[:], pt[:], biasr[:])
        e0.append(e)

    ttc.close()

    ffnw = _Pool(tc, name="ffnw", bufs=1)
    w1_t = ffnw.tile([P, ND, FF], F32, tag="w1_t")
    nc.sync.dma_start(w1_t[:], io["w1T"].rearrange("(a b) c -> b a c", b=P))
    w2_t = ffnw.tile([P, NF, D], F32, tag="w2_t")
    nc.sync.dma_start(w2_t[:], io["w2T"].rearrange("(a b) c -> b a c", b=P))

    ainvT = []
    for i in range(NT):
        u1 = mt[(i, i)]
        ptv = psmall()
        nc.tensor.transpose(ptv[:], u1[:], id_t[:])
        v1 = ttmp.tile([P, P], F32, tag="inv_v1")
        nc.vector.tensor_copy(v1[:], ptv[:])
        us, vs = {1: u1}, {1: v1}
        k = 1
        while k < 64:
            ptu = psmall()
            nc.tensor.matmul(ptu[:], _mm(vs[k][:]), _mm(us[k][:]),
                             start=True, stop=True)
            u2 = ttmp.tile([P, P], F32, tag=f"inv_u{2 * k}")
            nc.vector.tensor_copy(u2[:], ptu[:])
            ptv2 = psmall()
            nc.tensor.matmul(ptv2[:], _mm(us[k][:]), _mm(vs[k][:]),
                             start=True, stop=True)
            v2 = ttmp.tile([P, P], F32, tag=f"inv_v{2 * k}")
            nc.vector.tensor_copy(v2[:], ptv2[:])
            us[2 * k], vs[2 * k] = u2, v2
            k *= 2
        pcur = ttt.tile([P, P], F32, tag=f"ainvT{i}")
        nc.vector.tensor_sub(pcur[:], id_t[:], u1[:])
        for j in range(1, 7):
            ptp_ = psmall()
            nc.tensor.matmul(ptp_[:], _mm(vs[2 ** j][:]), _mm(pcur[:]),
                             start=True, stop=True)
            nc.vector.tensor_add(pcur[:], pcur[:], ptp_[:])
        ainvT.append(pcur)

    e_t = []
    for i in range(NT):
        if i > 0:
            pt = psmall()
            for j in range(i):
                nc.tensor.matmul(pt[:], _mm(mt[(j, i)][:]), _mm(e_t[j][:]),
                                 start=(j == 0), stop=(j == i - 1))
            nc.vector.tensor_sub(e0[i][:], e0[i][:], pt[:])
        pt2 = psmall()
        nc.tensor.matmul(pt2[:], _mm(ainvT[i][:]), _mm(e0[i][:]),
                         start=True, stop=True)
        ei = ttt.tile([P, P], F32, tag=f"e_{i}")
        nc.vector.tensor_copy(ei[:], pt2[:])
        e_t.append(ei)

    hxs_t = ttt.tile([P, ND, P], F32, tag="hxs_t")
    nc.sync.dma_start(hxs_t[:], io["hx_sel"].rearrange("(a b) c -> b a c", b=P))
    g2full = [ttt.tile([P, D], F32, tag=f"g2f{i}") for i in range(NT)]
    for i in range(NT):
        nc.vector.tensor_add(g2full[i][:], h1[i][:], x2[i][:])
    g2T = [ttt.tile([P, T], F32, tag=f"g2T{i}") for i in range(ND)]
    transpose_to(g2T, g2full)
    ag_in = dram.tile([T, P], F32, tag="ag_in")
    for ti in range(NT):
        pt = psmall()
        for dt_ in range(ND):
            nc.tensor.matmul(pt[:], _mm(g2T[dt_][:, ti * P:(ti + 1) * P]),
                             _mm(hxs_t[:, dt_]),
                             start=(dt_ == 0), stop=(dt_ == ND - 1))
        h2s = ttmp.tile([P, P], F32, tag="h2s")
        nc.vector.tensor_add(h2s[:], pt[:], e_t[ti][:])
        nc.sync.dma_start(ag_in[ti * P:(ti + 1) * P, :], h2s[:])

    ag_out = dram.tile([ND, T, P], F32, tag="ag_out")
    nc.gpsimd.collective_compute(
        "AllGather", ALU.bypass, replica_groups=GROUPS,
        ins=[ag_in.opt()], outs=[ag_out.opt()])


    ffn = _Pool(tc, name="ffn", bufs=1)
    ftmp = _Pool(tc, name="ftmp", bufs=2)

    tsel_t = ffn.tile([P, NT, P], F32, tag="tsel_t")
    nc.sync.dma_start(tsel_t[:], io["tsel"].rearrange("(a b) c -> b a c", b=P))
    h2T = [ffn.tile([P, P], F32, tag=f"h2T{r}") for r in range(ND)]
    for r in range(ND):
        agr = ftmp.tile([P, NT, P], F32, tag="agr")
        nc.sync.dma_start(agr[:],
                          ag_out[r].rearrange("(a b) c -> b a c", b=P))
        pt = psmall()
        for tt_ in range(NT):
            nc.tensor.matmul(pt[:], _mm(agr[:, tt_]), _mm(tsel_t[:, tt_]),
                             start=(tt_ == 0), stop=(tt_ == NT - 1))
        nc.vector.tensor_copy(h2T[r][:], pt[:])

    h2j = ffn.tile([P, D], F32, tag="h2j")
    pt = pbig()
    for r in range(ND):
        nc.tensor.transpose(pt[:, r * P:(r + 1) * P], h2T[r][:], id_t[:])
    nc.vector.tensor_copy(h2j[:], pt[:])

    z3 = ffn.tile([P, D], F32, tag="z3")
    layernorm_tile(z3, h2j, gb[4], gb[5], ftmp)
    zT = ffn.tile([P, ND, P], F32, tag="zT")
    ptz = pbig()
    for dt_ in range(ND):
        nc.tensor.transpose(ptz[:, dt_ * P:(dt_ + 1) * P],
                            z3[:, dt_ * P:(dt_ + 1) * P], id_t[:])
    nc.vector.tensor_copy(zT.rearrange("a b c -> a (b c)"), ptz[:])

    b1_t = ffn.tile([P, NF], F32, tag="b1_t")
    nc.sync.dma_start(b1_t[:], io["b1c"][:])
    g_t = [ffn.tile([P, P], F32, tag=f"gff{fc}") for fc in range(NF)]
    for fc in range(NF):
        pt = psmall()
        for dt_ in range(ND):
            nc.tensor.matmul(pt[:], _mm(w1_t[:, dt_, fc * P:(fc + 1) * P]),
                             _mm(zT[:, dt_]),
                             start=(dt_ == 0), stop=(dt_ == ND - 1))
        nc.scalar.activation(out=g_t[fc][:], in_=pt[:], func=AF.Gelu,
                             bias=b1_t[:, fc:fc + 1], scale=1.0)

    ptf = pbig()
    for fc in range(NF):
        nc.tensor.matmul(ptf[:], _mm(g_t[fc][:]), _mm(w2_t[:, fc]),
                         start=(fc == 0), stop=(fc == NF - 1))
    b2_t = ffn.tile([P, D], F32, tag="b2_t")
    nc.sync.dma_start(b2_t[:], io["b2"][0:1, :].to_broadcast((P, D)))
    o_sb = ffn.tile([P, D], F32, tag="o_sb")
    nc.vector.tensor_add(o_sb[:], ptf[:], h2j[:])
    nc.vector.tensor_add(o_sb[:], o_sb[:], b2_t[:])
    nc.sync.dma_start(io["out"][:], o_sb[:])

    for pool in (ftmp, ffn, ffnw, ttmp, ttt, dram, psB, psA,
                 persist):
        pool.close()


_CACHE = {}


def _host_constants():
    nb = NOV_BETA
    t_idx = np.arange(T)
    dt_ = t_idx[:, None] - t_idx[None, :]
    dpow = np.maximum(dt_, 0)
    aema = np.where(dt_ >= 1, (1 - nb) * nb ** np.maximum(dt_ - 1, 0), 0.0)
    fold = (1 - MOM_BETA) / D
    alrm = fold * np.where(dt_ >= 0, (1 - nb) * nb ** dpow, 0.0)
    c0v = (fold * nb ** (t_idx + 1)).reshape(T, 1)
    bmat = np.where(dt_ >= 0, MOM_BETA ** dpow, 0.0)
    lstrict = (dt_ >= 1).astype(np.float64)
    p_idx = np.arange(P)
    dp = p_idx[:, None] - p_idx[None, :]
    return dict(
        aemaT=aema.T.astype(np.float32).copy(),
        alrmT=alrm.T.astype(np.float32).copy(),
        c0=c0v.astype(np.float32),
        bmat=bmat.astype(np.float32),
        lstrictT=lstrict.T.astype(np.float32).copy(),
        triu01=(dp < 0).astype(np.float32),
        idmat=np.eye(P, dtype=np.float32),
        maskdiag=np.where(dp >= 0, 0.0, NEG).astype(np.float32),
    )


def _build_in_maps(inputs):
    if "consts" not in _CACHE:
        _CACHE["consts"] = _host_constants()
    consts = _CACHE["consts"]
    wqT = inputs["Wq"].T
    wk1T = inputs["Wk1"].T
    wk2T = inputs["Wk2"].T
    wvT = inputs["Wv"].T
    woT = inputs["Wo"].T
    baseT = inputs["base_weight"].T
    w1T = np.ascontiguousarray(inputs["W1"].T)
    w2T = np.ascontiguousarray(inputs["W2"].T)
    lngb = np.stack([inputs["ln1_g"], inputs["ln1_b"], inputs["ln2_g"],
                     inputs["ln2_b"], inputs["ln3_g"], inputs["ln3_b"]])
    b1c = np.ascontiguousarray(inputs["b1"].reshape(NF, P).T)
    in_maps = []
    for c in range(N_CORES):
        b, j = divmod(c, 4)
        sl = slice(j * P, (j + 1) * P)
        basep = np.ascontiguousarray(baseT[:, sl])
        basep[sl, :] -= np.eye(P, dtype=np.float32)
        hx_sel = np.zeros((D, P), np.float32)
        hx_sel[sl, :] = np.eye(P, dtype=np.float32)
        tsel = np.zeros((T, P), np.float32)
        tsel[sl, :] = np.eye(P, dtype=np.float32)
        m = dict(
            x=inputs["x"][b],
            wqT=wqT[:, sl], wk1T=wk1T[:, sl], wk2T=wk2T[:, sl],
            wvT=wvT[:, sl], woT=woT[sl, :],
            lam_l=inputs["lambda_logit"][2 * j:2 * j + 2].reshape(1, 2),
            lngb=lngb,
            baseT=basep,
            bias_sl=inputs["ttt_bias"][sl].reshape(1, P),
            loglr=np.asarray(inputs["log_inner_lr"]).reshape(1, 1),
            hx_sel=hx_sel, tsel=tsel,
            w1T=w1T, b1c=b1c, w2T=w2T,
            b2=inputs["b2"].reshape(1, D),
            **consts,
        )
        in_maps.append({k: np.ascontiguousarray(v, dtype=np.float32)
                        for k, v in m.items()})
    return in_maps


def get_nc():
    if "nc" not in _CACHE:
        _CACHE["nc"] = build_nc()
    return _CACHE["nc"]


def kernel(**inputs):
    inputs = {k: np.asarray(v) for k, v in inputs.items()}
    nc = get_nc()
    in_maps = _build_in_maps(inputs)
    res = run_bass_kernel_spmd(nc, in_maps, list(range(N_CORES)))
    outs = res.results
    full = np.zeros((B, T, D), np.float32)
    for c in range(N_CORES):
        b, j = divmod(c, 4)
        full[b, j * P:(j + 1) * P, :] = outs[c]["out"]
    return full
